# revision 1
# baseline (speedup 1.0000x reference)
"""GCN (3-layer message passing) distributed over 8 TRN2 NeuronCores.

Sharding: nodes split evenly across 8 cores (rows). Weights replicated.
Per layer: local matmul h = x @ W + b (node-major out via x^T-stationary
matmuls), then a CHUNKED AllGather of the h table: hloc row-range chunks
are AllGathered as soon as the matmul has produced them, so chunk c's
exchange (on TOPSP/SDMA silicon) overlaps the matmul for chunk c+1.
Each core then runs the local segment-sum over its incoming edges as
one-hot matmuls (edges tiled 128 at a time, PSUM-accumulated per
128-node destination window), gathering source rows straight from the
allgathered table; bias/relu fused in the epilogue.

The allgathered table has NC*NPAD = 50176 rows, above the int16 gather
index ceiling, so each window's edges are split into two half-table
groups (chunks 0-1 / 2-3), gathered separately with rebased indices.
This also lets the first group's gathers start before the last chunks
arrive.

Layer 3's inclusion linear Wi is folded into W3 (segment_sum commutes
with right-matmul), so the last exchange is only 16 (padded 128) wide.

Everything data-dependent (edge tiling) is computed host-side in
preprocess(); all 8 cores run one SPMD graph whose shapes depend only
on those computed constants.
"""
import sys

sys.path.insert(0, "/opt/trn_rl_repo")

import numpy as np
import ml_dtypes

import concourse.bass as bass
import concourse.bacc as bacc
import concourse.mybir as mybir
import concourse.tile as tile
from concourse.bass_utils import run_bass_kernel_spmd

NC = 8
C = 2       # AllGather chunks per layer
NG = 2      # gather groups (half-tables) per window
BF16 = mybir.dt.bfloat16
F32 = mybir.dt.float32
I16 = mybir.dt.int16

last_exec_time_ns = None
last_results = None


def _wrap16(idx, ncols):
    """[n] int -> [128, n/16] int16 wrapped (idx i at [i%16, i//16]) and
    replicated to 128 partitions."""
    a = np.asarray(idx, np.int16).reshape(ncols, 16).T  # [16, n/16]
    return np.tile(a, (8, 1))


def preprocess(features, W1, b1, W2, b2, W3, b3, Wi, bi, src, dst,
               skip_pads=True):
    """Host-side sharding/setup. Returns (cfg dict, in_maps list).

    skip_pads: pad rows of each aggregation window get idx -1 and are
    skipped via num_idxs_reg (the matmul one-hot zeros them out; gather
    slots are pre-zeroed so stale lanes stay finite)."""
    N, K1t = features.shape  # 50000, 1433
    E = src.shape[0]
    assert N % NC == 0
    NLOC = N // NC
    MBLK = (NLOC + 127) // 128
    NPAD = MBLK * 128
    assert NPAD % C == 0
    CH = NPAD // C              # hloc rows per AllGather chunk
    GR = (C // NG) * NC * CH    # recv rows per gather group (half-table)
    assert GR <= 32768

    TW = [768, 512, 128]          # h-table widths (bf16, 256B-aligned)
    K = [1536, TW[0], TW[1]]      # matmul contraction dims (128-aligned)
    KB = [k // 128 for k in K]

    # ---- weights (fold Wi into W3), padded, bf16 ----
    W3f = (W3.astype(np.float64) @ Wi.astype(np.float64)).astype(np.float32)
    b3f = (b3.astype(np.float64) @ Wi.astype(np.float64)).astype(np.float32)

    def pad2(a, r, c):
        out = np.zeros((r, c), np.float32)
        out[: a.shape[0], : a.shape[1]] = a
        return out

    w1 = pad2(W1, K[0], TW[0]).astype(ml_dtypes.bfloat16)
    w2 = pad2(W2, K[1], TW[1]).astype(ml_dtypes.bfloat16)
    w3 = pad2(W3f, K[2], TW[2]).astype(ml_dtypes.bfloat16)
    b1p = np.tile(pad2(b1[None, :], 1, TW[0]), (128, 1))
    b2p = np.tile(pad2(b2[None, :], 1, TW[1]), (128, 1))
    b3p = np.tile(pad2(b3f[None, :], 1, TW[2]), (128, 1))
    bip = np.tile(pad2(bi[None, :], 1, TW[2]), (128, 1))

    # ---- per-core transposed features [K[0], NPAD] bf16 ----
    featTs = []
    for c in range(NC):
        ft = np.zeros((K[0], NPAD), np.float32)
        ft[:K1t, :NLOC] = features[c * NLOC : (c + 1) * NLOC].T
        featTs.append(ft.astype(ml_dtypes.bfloat16))

    # ---- graph structure ----
    src = np.asarray(src, np.int64)
    dst = np.asarray(dst, np.int64)
    owner = src // NLOC
    dcore = dst // NLOC

    # recv row of a source node (owner o, local row r), chunk-major:
    #   chunk = r // CH; row = chunk*NC*CH + o*CH + r % CH
    # group g = chunk // (C//NG); gather indices are rebased by g*GR.
    # Edge tiling per dest core: within each 128-dst window, edges are
    # split by group, each group's edges padded to whole 128-edge tiles
    # (tile counts shared across cores = max).
    per_core = []
    for d in range(NC):
        m = dcore == d
        es, ed = src[m], dst[m]
        eo = es // NLOC
        r = es - eo * NLOC
        ch = r // CH
        tbl = ch * (NC * CH) + eo * CH + (r - ch * CH)
        grp = ch // (C // NG)
        ldst = ed - d * NLOC
        win = ldst // 128
        rel = ldst % 128
        per_core.append((tbl, grp, win, rel))

    # common tiles-per-(window,group) across cores
    NW = MBLK
    Twg = np.zeros((NW, NG), np.int64)
    for d in range(NC):
        _, grp, win, _ = per_core[d]
        for g in range(NG):
            cnt = np.bincount(win[grp == g], minlength=NW)
            Twg[:, g] = np.maximum(Twg[:, g], (cnt + 127) // 128)
    Twg[0, 0] = max(Twg[0, 0], 1)  # keep at least one tile overall
    Tw = Twg.sum(axis=1)
    tile_start = np.concatenate([[0], np.cumsum(Tw)])  # per window
    # group g of window w starts at tile_start[w] + (g>0)*Twg[w,0]
    T8 = ((int(tile_start[-1]) + 7) // 8) * 8

    gidxs, dstrels, acnts = [], [], []
    for d in range(NC):
        tbl, grp, win, rel = per_core[d]
        order = np.lexsort((tbl, grp, win))
        tbl, grp, win, rel = tbl[order], grp[order], win[order], rel[order]
        gi = np.full(T8 * 128, -1 if skip_pads else 0, np.int64)
        r = np.full(T8 * 128, -1.0, np.float32)
        cnts = np.zeros((NW, NG), np.int32)
        for w in range(NW):
            mw = win == w
            off = int(tile_start[w]) * 128
            for g in range(NG):
                mg = mw & (grp == g)
                n = int(mg.sum())
                cnts[w, g] = n
                gi[off : off + n] = tbl[mg] - g * GR
                r[off : off + n] = rel[mg]
                off += int(Twg[w, g]) * 128
        gidxs.append(_wrap16(gi, T8 * 8))
        dstrels.append(np.ascontiguousarray(r.reshape(T8, 128).T))
        acnts.append(cnts.reshape(1, NW * NG))

    iota = np.tile(np.arange(128, dtype=np.float32)[None, :], (128, 1))

    cfg = dict(NLOC=NLOC, NPAD=NPAD, MBLK=MBLK, TW=TW, K=K, KB=KB,
               CH=CH, GR=GR,
               Twg=[[int(x) for x in row] for row in Twg],
               tile_start=[int(x) for x in tile_start],
               T8=T8, OUT_W=16)

    in_maps = []
    for c in range(NC):
        in_maps.append({
            "featT": featTs[c],
            "w1": w1, "w2": w2, "w3": w3,
            "b1": b1p, "b2": b2p, "b3": b3p, "bi": bip,
            "gidx": gidxs[c], "drel": dstrels[c],
            "acnt": acnts[c], "iota": iota,
        })
    return cfg, in_maps


def build(cfg, nq=4, reps=1, queue_plan=None, collect_gathers=None):
    NLOC, NPAD, MBLK = cfg["NLOC"], cfg["NPAD"], cfg["MBLK"]
    TW, K, KB = cfg["TW"], cfg["K"], cfg["KB"]
    CH, GR = cfg["CH"], cfg["GR"]
    Twg, tile_start, T8 = cfg["Twg"], cfg["tile_start"], cfg["T8"]
    OUT_W = cfg["OUT_W"]
    NW = MBLK

    AGP_BUFS = 7
    nc = bacc.Bacc("TRN2", target_bir_lowering=False, debug=False,
                   num_devices=NC, num_swdge_queues=nq)

    featT = nc.declare_dram_parameter("featT", [K[0], NPAD], BF16, isOutput=False)
    wts = [nc.declare_dram_parameter(f"w{l+1}", [K[l], TW[l]], BF16, isOutput=False)
           for l in range(3)]
    bs = [nc.declare_dram_parameter(f"b{l+1}", [128, TW[l]], F32, isOutput=False)
          for l in range(3)]
    bi = nc.declare_dram_parameter("bi", [128, TW[2]], F32, isOutput=False)
    gidx = nc.declare_dram_parameter("gidx", [128, T8 * 8], I16, isOutput=False)
    drel = nc.declare_dram_parameter("drel", [128, T8], F32, isOutput=False)
    acnt = nc.declare_dram_parameter("acnt", [1, NW * NG], mybir.dt.int32,
                                     isOutput=False)
    iota = nc.declare_dram_parameter("iota", [128, 128], F32, isOutput=False)
    out = nc.declare_dram_parameter("out", [NLOC, OUT_W], F32, isOutput=True)

    hloc = [nc.dram_tensor(f"hloc{l}", [NPAD, TW[l]], BF16) for l in range(3)]
    recv = [nc.dram_tensor(f"recv{l}", [NC * NPAD, TW[l]], BF16,
                           addr_space="Shared") for l in range(3)]
    xs = [None, nc.dram_tensor("x2", [NPAD, TW[0]], BF16),
          nc.dram_tensor("x3", [NPAD, TW[1]], BF16)]

    groups = [list(range(NC))]

    with tile.TileContext(nc) as tc:
        with (
            tc.tile_pool(name="wpool", bufs=1) as wpool,
            tc.tile_pool(name="bpool", bufs=1) as bpool,
            tc.tile_pool(name="ipool", bufs=1) as ipool,
            tc.tile_pool(name="xtp", bufs=2) as xtp,
            tc.tile_pool(name="mmpsum", bufs=2, space="PSUM") as mmpsum,
            tc.tile_pool(name="hbp", bufs=3) as hbp,
            tc.tile_pool(name="agp", bufs=AGP_BUFS) as agp,
            tc.tile_pool(name="ohp", bufs=6) as ohp,
            tc.tile_pool(name="apsum", bufs=2, space="PSUM") as apsum,
            tc.tile_pool(name="xop", bufs=3) as xop,
        ):
            # resident: indices, iota, dstrel
            gidx_t = ipool.tile([128, T8 * 8], I16, tag="gidx")
            nc.sync.dma_start(gidx_t[:], gidx[:])
            drel_t = ipool.tile([128, T8], F32, tag="drel")
            nc.sync.dma_start(drel_t[:], drel[:])
            iota_t = ipool.tile([128, 128], F32, tag="iota")
            nc.sync.dma_start(iota_t[:], iota[:])
            obuf = ipool.tile([128, NW, OUT_W], F32, tag="obuf")
            acnt_t = ipool.tile([1, NW * NG], mybir.dt.int32, tag="acnt")
            nc.sync.dma_start(acnt_t[:], acnt[:])
            TWMAX = max(a + b for (a, b) in Twg)
            # zero the gather slots once so rows skipped by short gathers
            # (num_idxs_reg < num_idxs) read as finite values
            for _ in range(AGP_BUFS):
                zt = agp.tile([128, TWMAX, max(TW)], BF16, tag="ag")
                nc.vector.memset(zt[:], 0.0)
            nreg_a = nc.gpsimd.alloc_register()
            nreg_b = nc.gpsimd.alloc_register()
            nreg_c = nc.gpsimd.alloc_register()
            nreg_d = nc.gpsimd.alloc_register()
            nregs = [nreg_a, nreg_b, nreg_c, nreg_d]
            gcall = 0

            for _rep in range(reps):
             for l in range(3):
              with nc.named_scope(f"L{l}"):
                  # ---- resident weights/bias for this layer ----
                  wt = wpool.tile([128, KB[l], TW[l]], BF16, tag="w")
                  nc.sync.dma_start(
                      wt[:], wts[l].rearrange("(kb p) w -> p kb w", p=128))
                  bt = bpool.tile([128, TW[l]], F32, tag="b")
                  nc.sync.dma_start(bt[:], bs[l][:])
                  if l == 2:
                      bit = bpool.tile([128, TW[2]], F32, tag="bi")
                      nc.sync.dma_start(bit[:], bi[:])

                  # ---- matmul: h = x @ W + b  (node-major PSUM out) ----
                  nslices = [(s, min(s + 512, TW[l])) for s in range(0, TW[l], 512)]
                  sc_mm = nc.enter_named_scope(f"mm{l}", False)[0]
                  NRW = 512
                  for nr in range(0, NPAD, NRW):
                      rw = min(NRW, NPAD - nr)
                      stripes = []
                      for kb in range(KB[l]):
                          st = xtp.tile([128, NRW], BF16, tag=f"xt{kb}")
                          if l == 0:
                              nc.sync.dma_start(
                                  st[:, :rw],
                                  featT[kb * 128 : (kb + 1) * 128, nr : nr + rw])
                          else:
                              nc.sync.dma_start_transpose(
                                  st[:, :rw],
                                  xs[l][nr : nr + rw, kb * 128 : (kb + 1) * 128])
                          stripes.append(st)
                      for m in range(rw // 128):
                          ps = mmpsum.tile([128, TW[l]], F32, tag="mmps")
                          for kb in range(KB[l]):
                              for (s0, s1) in nslices:
                                  nc.tensor.matmul(
                                      ps[:, s0:s1],
                                      stripes[kb][:, m * 128 : (m + 1) * 128],
                                      wt[:, kb, s0:s1],
                                      start=(kb == 0), stop=(kb == KB[l] - 1))
                          hb = hbp.tile([128, TW[l]], BF16, tag="hb")
                          nc.vector.tensor_tensor(
                              hb[:], ps[:], bt[:], op=mybir.AluOpType.add)
                          nc.sync.dma_start(
                              hloc[l][nr + m * 128 : nr + (m + 1) * 128, :], hb[:])
                  nc.leave_named_scope(f"mm{l}", sc_mm, False)

                  # ---- chunked AllGather exchange ----
                  sc_sg = nc.enter_named_scope(f"ag{l}", False)[0]
                  for c in range(C):
                      nc.gpsimd.collective_compute(
                          "AllGather", mybir.AluOpType.bypass,
                          replica_groups=groups,
                          ins=[hloc[l][c * CH : (c + 1) * CH, :]],
                          outs=[recv[l][c * NC * CH : (c + 1) * NC * CH, :]])
                  nc.leave_named_scope(f"ag{l}", sc_sg, False)

                  # ---- aggregation: segment-sum via one-hot matmuls ----
                  sc_ag = nc.enter_named_scope(f"agg{l}", False)[0]
                  for w in range(NW):
                      ps = apsum.tile([128, TW[l]], F32, tag="aps")
                      t0 = tile_start[w]
                      ntile = Twg[w][0] + Twg[w][1]
                      gt = agp.tile([128, TWMAX, TW[l]], BF16, tag="ag")
                      goff = 0
                      for g in range(NG):
                          twg = Twg[w][g]
                          if twg == 0:
                              continue
                          nreg = nregs[gcall % 4]
                          nc.gpsimd.reg_load(
                              nreg, acnt_t[0:1, w * NG + g : w * NG + g + 1])
                          gq = queue_plan[gcall] if queue_plan else 0
                          gi_ = nc.gpsimd.dma_gather(
                              gt[:, goff : goff + twg, :],
                              recv[l][g * GR : (g + 1) * GR],
                              gidx_t[:, (t0 + goff) * 8 : (t0 + goff + twg) * 8],
                              twg * 128, nreg, TW[l], queue_num=gq)
                          if collect_gathers is not None:
                              collect_gathers.append(gi_)
                          gcall += 1
                          goff += twg
                      for tl in range(ntile):
                          t = t0 + tl
                          oh = ohp.tile([128, 128], BF16, tag="oh")
                          nc.vector.tensor_scalar(
                              oh[:], iota_t[:], drel_t[:, t : t + 1], None,
                              mybir.AluOpType.is_equal)
                          rhs = gt[:, tl, :]
                          for (s0, s1) in nslices:
                              nc.tensor.matmul(
                                  ps[:, s0:s1], oh[:], rhs[:, s0:s1],
                                  start=(tl == 0), stop=(tl == ntile - 1))
                      # ---- epilogue ----
                      if l < 2:
                          xb = xop.tile([128, TW[l]], BF16, tag="xo")
                          nc.vector.tensor_scalar_max(xb[:], ps[:], 0.0)
                          nc.sync.dma_start(
                              xs[l + 1][w * 128 : (w + 1) * 128, :], xb[:])
                      else:
                          nc.vector.tensor_tensor(
                              obuf[:, w, :], ps[:, :OUT_W], bit[:, :OUT_W],
                              op=mybir.AluOpType.add)
                          nc.vector.tensor_scalar_max(
                              obuf[:, w, :], obuf[:, w, :], 0.0)
                  if l == 2:
                      # one batched store for the full windows, then the tail
                      WFULL = NLOC // 128
                      nc.sync.dma_start(
                          out[: WFULL * 128, :]
                          .rearrange("(w p) c -> p w c", p=128),
                          obuf[:, :WFULL, :])
                      rows = NLOC - WFULL * 128
                      if rows > 0:
                          nc.sync.dma_start(
                              out[WFULL * 128 :, :], obuf[:rows, WFULL, :])
                  nc.leave_named_scope(f"agg{l}", sc_ag, False)
    nc.finalize()
    return nc


DMASW0_IDX = 11  # PROC_NAME_TO_IDX["DMASW0"]


def build_lane_matched(cfg, reps=1):
    insts = []
    build(cfg, nq=4, reps=reps, collect_gathers=insts)
    lanes = [((bi.ins.bass_scheduled_proc - DMASW0_IDX) % 4) for bi in insts]
    insts2 = []
    nc = build(cfg, nq=4, reps=reps, queue_plan=lanes, collect_gathers=insts2)
    lanes2 = [((bi.ins.bass_scheduled_proc - DMASW0_IDX) % 4) for bi in insts2]
    assert lanes2 == lanes, "lane assignment changed between passes"
    return nc


def kernel(**inputs):
    global last_exec_time_ns, last_results
    inputs = {k: np.asarray(v) for k, v in inputs.items()}
    cfg, in_maps = preprocess(**inputs)
    nc = build_lane_matched(cfg)
    res = None
    # trace=True needs the axon NTFF hook; fall back to untraced runs, and
    # retry once more on transient device errors (NRT_EXEC_UNIT_UNRECOVERABLE).
    for attempt, trace in enumerate([True, False, False]):
        try:
            res = run_bass_kernel_spmd(
                nc, in_maps, core_ids=list(range(NC)), trace=trace)
            break
        except Exception:
            if attempt == 2:
                raise
            import time
            time.sleep(15)
    last_exec_time_ns = res.exec_time_ns
    last_results = res
    return np.concatenate([res.results[c]["out"] for c in range(NC)], axis=0)

